# revision 1
# baseline (speedup 1.0000x reference)
"""Trainium2 Bass kernel for the Haar-mask MLP (histogram_binning).

Key algorithmic fact: every Haar interval edge is a multiple of 2^-10, so the
reference's masks -- and therefore the entire MLP output -- depend only on
u = floor(t * 1024) (1024 possible values, exact in fp32 since *1024 is a
power-of-two scale).  The whole network collapses to a 1024x3 lookup table,
computed once on host from the tiny weights.  The device work is the
memory-bound part: stream t, compute u, gather LUT[u], stream out.

Device plan (pure data parallel, 8 cores, 16384 elements each):
  - DMA t chunk into SBUF [128p x 128] (host pre-permutes so that partition
    16c+p, slot s holds element 2048c + 16s + p -- exactly the wrapped index
    layout the GpSimd gathers want).
  - u = floor(t*1024) on DVE (exact under any convert rounding mode),
    convert+clamp+scale to a 16-bit index.
  - Replicate the LUT per partition; GpSimd gather per chunk; DMA back.

Gather impl variants (GATHER_IMPL):
  ap3: ap_gather, d=3 rows           -- table [1024,3]/partition, out interleaved
  ic3: indirect_copy, inner=3, idx*3 -- same layout, resident HW-assisted op
  ap1: ap_gather, d=1, planar table  -- partition p holds LUT[:, p%16%3]
  ic1: indirect_copy, inner=1, planar
The *3 variants DMA partitions {16c} (rows of 512x3); the *1 variants DMA the
full tile and the host picks rows {16c+f}.
"""

from contextlib import ExitStack

import numpy as np

import concourse.tile as tile
from concourse import bacc, mybir
from concourse.bass_utils import run_bass_kernel_spmd

N_CORES = 8
B, T, F = 16, 8192, 3
N = B * T                    # 131072 total elements
NPC = N // N_CORES           # 16384 per neuron core
P = 128                      # SBUF partitions
S = NPC // P                 # 128 slots per partition
NBINS = 1024
NCHUNK = 4                   # gather/store pipeline chunks
IDXS = NPC // 8 // NCHUNK    # 512 indices per q7-core per chunk

GATHER_IMPL = "ic1"
RUN_KWARGS = {}              # test harness may set {"trace": True}
LAST_RESULTS = None
_CACHE = {}


def _build_lut(W1, b1, W2, b2, W3, b3):
    """MLP output for each of the 1024 half-interval bins, fp32 math."""
    u = np.arange(NBINS)
    acc = np.zeros((NBINS, W1.shape[1]), np.float32)
    for j in range(10):
        k = u >> (10 - j)                       # floor(t * 2^j) for t in bin u
        idx = (1 << j) - 1 + k                  # level-j block offset + k
        sign = np.where((u >> (9 - j)) & 1 == 0, np.float32(1), np.float32(-1))
        acc = acc + sign[:, None] * W1[idx]
    h = np.maximum(acc + b1, np.float32(0))
    h = np.maximum(h @ W2 + b2, np.float32(0))
    return (h @ W3 + b3).astype(np.float32)     # (1024, 3)


def _build_nc(impl):
    planar = impl.endswith("1")
    use_ic = impl.startswith("ic")
    row = NBINS if planar else NBINS * F        # table row elements/partition
    gw = IDXS if planar else IDXS * F           # gather out elements/partition

    nc = bacc.Bacc("TRN2", target_bir_lowering=False, debug=False,
                   enable_asserts=False, num_devices=N_CORES)
    f32 = mybir.dt.float32
    idt = mybir.dt.uint16 if use_ic else mybir.dt.int16
    t_d = nc.dram_tensor("t", [P, S], f32, kind="ExternalInput")
    lut_d = nc.dram_tensor("lut", [P, row], f32, kind="ExternalInput")
    if planar:
        out_d = nc.dram_tensor("out", [NCHUNK, P, IDXS], f32,
                               kind="ExternalOutput")
    else:
        out_d = nc.dram_tensor("out", [8, NCHUNK, IDXS * F], f32,
                               kind="ExternalOutput")

    with tile.TileContext(nc) as tc, ExitStack() as ctx:
        cpool = ctx.enter_context(tc.tile_pool(name="c", bufs=1))
        gpool = ctx.enter_context(tc.tile_pool(name="g", bufs=1))

        t_sb = cpool.tile([P, S], f32)
        nc.sync.dma_start(t_sb[:], t_d[:, :])

        # split the table broadcast across partition quarters AND across
        # engines, so each lands on its own HWDGE queue (the broadcast gates
        # the first gather; same-engine splits would serialize on one queue)
        tab = cpool.tile([P, row], f32)
        for q, eng in enumerate((nc.sync, nc.scalar, nc.sync, nc.scalar)):
            eng.dma_start(tab[q * 32:(q + 1) * 32, :],
                          lut_d[q * 32:(q + 1) * 32, :])

        # exact floor(t*1024): round-to-int (any rounding mode), then
        # subtract 1 wherever the rounded value exceeds the true value
        uf = cpool.tile([P, S], f32)
        ii = cpool.tile([P, S], mybir.dt.int32)
        fb = cpool.tile([P, S], f32)
        adj = cpool.tile([P, S], f32)
        ui = cpool.tile([P, S], f32)
        idx = cpool.tile([P, S], idt)
        nc.vector.tensor_scalar(uf[:], t_sb[:], 1024.0, None,
                                mybir.AluOpType.mult)
        nc.vector.tensor_copy(ii[:], uf[:])
        nc.vector.tensor_copy(fb[:], ii[:])
        nc.vector.tensor_tensor(adj[:], fb[:], uf[:], mybir.AluOpType.is_gt)
        nc.vector.tensor_sub(ui[:], fb[:], adj[:])
        if use_ic and not planar:               # scale idx by 3 for ranges
            mn = cpool.tile([P, S], f32)
            nc.vector.tensor_scalar(mn[:], ui[:], 1023.0, None,
                                    mybir.AluOpType.min)
            nc.vector.tensor_scalar(idx[:], mn[:], 3.0, None,
                                    mybir.AluOpType.mult)
        else:
            nc.vector.tensor_scalar(idx[:], ui[:], 1023.0, None,
                                    mybir.AluOpType.min)

        spc = S // NCHUNK                        # idx columns per chunk
        for k in range(NCHUNK):
            g = gpool.tile([P, gw], f32, tag=f"g{k}")
            idx_k = idx[:, k * spc:(k + 1) * spc]
            if use_ic:
                d = 1 if planar else F
                nc.gpsimd.indirect_copy(
                    g[:].rearrange("p (n d) -> p n d", d=d),
                    tab[:].rearrange("p (n d) -> p n d", d=d),
                    idx_k, i_know_ap_gather_is_preferred=True)
            else:
                nc.gpsimd.ap_gather(g[:], tab[:], idx_k,
                                    channels=P, num_elems=NBINS,
                                    d=1 if planar else F, num_idxs=IDXS)
            if planar:
                nc.sync.dma_start(out_d.ap()[k, :, :], g[:, :])
            else:
                nc.sync.dma_start(out_d.ap()[:, k, :], g[0:P:16, :])
    nc.compile()
    return nc


def _host_inputs(t, lut):
    planar = GATHER_IMPL.endswith("1")
    if planar:
        lut_rep = np.ascontiguousarray(lut.T[np.arange(P) % 16 % 3])
    else:
        lut_rep = np.ascontiguousarray(
            np.broadcast_to(lut.reshape(-1), (P, NBINS * F)))
    tf = np.ascontiguousarray(np.asarray(t, np.float32)).reshape(-1)
    # SBUF partition 16c+p slot s <- element 2048c + 16s + p of the core chunk
    tperm = (tf.reshape(N_CORES, 8, S, 16).transpose(0, 1, 3, 2)
             .reshape(N_CORES, P, S))
    return tperm, lut_rep


def _host_output(raw):
    """Per-core device output -> (NPC, 3)."""
    if GATHER_IMPL.endswith("1"):
        # raw [NCHUNK, 128, IDXS]; feature f of element (c, 512k+i) is at
        # [k, 16c+f, i]
        r = raw.reshape(NCHUNK, 8, 16, IDXS)[:, :, :F, :]   # k c f i
        return np.ascontiguousarray(r.transpose(1, 0, 3, 2)).reshape(NPC, F)
    # raw [8, NCHUNK, IDXS*F]: (c, k, i*3+f) -> element 2048c + 512k + i
    return raw.reshape(NPC, F)


def kernel(t, W1, b1, W2, b2, W3, b3):
    global LAST_RESULTS
    key = ("nc", GATHER_IMPL)
    if key not in _CACHE:
        _CACHE[key] = _build_nc(GATHER_IMPL)
    nc = _CACHE[key]

    lut = _build_lut(np.asarray(W1, np.float32), np.asarray(b1, np.float32),
                     np.asarray(W2, np.float32), np.asarray(b2, np.float32),
                     np.asarray(W3, np.float32), np.asarray(b3, np.float32))
    tperm, lut_rep = _host_inputs(t, lut)
    in_maps = [{"t": np.ascontiguousarray(tperm[m]), "lut": lut_rep}
               for m in range(N_CORES)]

    res = run_bass_kernel_spmd(nc, in_maps, list(range(N_CORES)), **RUN_KWARGS)
    LAST_RESULTS = res
    outs = [_host_output(res.results[m]["out"]) for m in range(N_CORES)]
    return np.concatenate(outs, axis=0).reshape(B, T, F).astype(np.float32)



# revision 5
# speedup vs baseline: 1.1258x; 1.1258x over previous
"""Trainium2 Bass kernel for the Haar-mask MLP (histogram_binning).

Every Haar interval edge is a multiple of 2^-10, so the reference's masks --
and therefore the entire MLP output -- depend only on u = floor(t * 1024)
(exact in fp32).  The network collapses to a 1024x3 lookup table computed on
host from the tiny weights; the device work is: stream t, compute u, gather
LUT[u], stream out.

Gather engine: SWDGE dma_gather.  The Q7 cores of one core-pair per queue
generate SDMA descriptors (16 gather packets each); the SDMA engines then
pull 16-byte LUT rows from HBM into SBUF at ~7.6 ns/element/queue, 4 queues
in parallel.  This beats the old gpsimd indirect_copy path (~34 Q7
cycles/element = 57 us) by ~2x even including the mlp-library IRAM load.

Layouts per core (16384 elements, j = element ordinal):
  t_d  [128, 1024] f32: t[j] at partition 16g + j%16 (all 8 groups g),
       column j//16 -- the wrapped index layout dma_gather's tx cores read,
       replicated per 16-partition group.
  idx  int16 [128, 1024] computed on DVE in the same layout.
  dst  [128, 128, 4] f32: gather writes element j at [j%128, j//128].
  out  [128, 128, 3] f32 (host reads element j from [j%128, j//128]).
"""

from contextlib import ExitStack

import numpy as np

import concourse.tile as tile
from concourse import bacc, mybir
from concourse.bass_utils import run_bass_kernel_spmd
from concourse.library_config import mlp as mlp_lib

N_CORES = 8
B, T, F = 16, 8192, 3
N = B * T                    # 131072 total elements
NPC = N // N_CORES           # 16384 per neuron core
P = 128                      # SBUF partitions
S = NPC // P                 # 128 slots per partition
NBINS = 1024
ROWW = 64                    # LUT row stride: 64 f32 = 256 B (SDMA stride unit)
GE = 4                       # gathered f32 per element (16 B payload)
NQ = 4                       # SWDGE queues (ucode max)
IPI = 1024                   # indices per dma_gather (ring is 128 descs)

IMPL = "dg16"                # dg<n>: n dma_gather instructions; 'x' = exact
RUN_KWARGS = {}              # test harness may set {"trace": True}
LAST_RESULTS = None
_CACHE = {}


def _build_lut(W1, b1, W2, b2, W3, b3):
    """MLP output for each of the 1024 half-interval bins, fp32 math."""
    u = np.arange(NBINS)
    acc = np.zeros((NBINS, W1.shape[1]), np.float32)
    for j in range(10):
        k = u >> (10 - j)                       # floor(t * 2^j) for t in bin u
        idx = (1 << j) - 1 + k                  # level-j block offset + k
        sign = np.where((u >> (9 - j)) & 1 == 0, np.float32(1), np.float32(-1))
        acc = acc + sign[:, None] * W1[idx]
    h = np.maximum(acc + b1, np.float32(0))
    h = np.maximum(h @ W2 + b2, np.float32(0))
    return (h @ W3 + b3).astype(np.float32)     # (1024, 3)


def _dma_gather_raw(gp, out_ap, in_ap, idxs_ap, num_idxs, elem_size, elem_step,
                    queue_num):
    """gpsimd.dma_gather minus the elem_size_bytes%256 assert (non-transpose
    HBM path: only the row STRIDE must be a 256B multiple, not the payload).
    Verified on hardware with 16B payloads."""
    _in_ap = gp.lower_ap_dma(in_ap, for_custom_bir_dma=True)
    return gp.add_instruction(mybir.InstDMAGatherAnt(
        name=gp.bass.get_next_instruction_name(),
        ins=[*_in_ap, gp.lower_ap(idxs_ap),
             gp.lower_val_access(gp.to_reg(num_idxs))],
        outs=[gp.lower_ap(out_ap)],
        transpose=False, num_idxs=num_idxs, elem_size=elem_size,
        stride_bytes_256=elem_step * 4 // 256, gen_mode=0, single_packet=True,
        queue_num=queue_num, sbuf_tokens_per_rank=0, sbuf_free_dim_per_rank=0,
        sbuf_free_dim_pad_per_rank=0, sbuf_byte_offset=0))


def _build_nc_dg(ninst, exact):
    nc = bacc.Bacc("TRN2", target_bir_lowering=False, debug=False,
                   enable_asserts=False, num_devices=N_CORES,
                   num_swdge_queues=NQ)
    f32 = mybir.dt.float32
    i16 = mybir.dt.int16
    cols = NPC // 16                             # 1024 idx columns
    t_d = nc.dram_tensor("t", [P, cols], f32, kind="ExternalInput")
    lut_d = nc.dram_tensor("lut", [NBINS, ROWW], f32, kind="ExternalInput")
    out_d = nc.dram_tensor("out", [P, S, F], f32, kind="ExternalOutput")

    ipc = NPC // ninst                           # indices per instruction
    assert ipc <= 2000, "SWDGE ring holds 128 descriptors (~2000 idx)"
    cpc = cols // ninst                          # idx columns per instruction
    spc = S // ninst                             # dst slots per instruction

    with tile.TileContext(nc) as tc, ExitStack() as ctx:
        cpool = ctx.enter_context(tc.tile_pool(name="c", bufs=1))
        qsems = [ctx.enter_context(nc.semaphore(f"q{q}")) for q in range(NQ)]

        # library IRAM load first: overlaps t DMA + index compute
        nc.gpsimd.load_library(mlp_lib)

        t_sb = cpool.tile([P, cols], f32)
        for q, eng in enumerate((nc.sync, nc.scalar, nc.sync, nc.scalar)):
            eng.dma_start(t_sb[q * 32:(q + 1) * 32, :],
                          t_d[q * 32:(q + 1) * 32, :])

        idx = cpool.tile([P, cols], i16)
        if exact:
            # exact floor(t*1024): round-to-int (any rounding mode), then
            # subtract 1 wherever the rounded value exceeds the true value
            uf = cpool.tile([P, cols], f32)
            ii = cpool.tile([P, cols], mybir.dt.int32)
            fb = cpool.tile([P, cols], f32)
            adj = cpool.tile([P, cols], f32)
            ui = cpool.tile([P, cols], f32)
            nc.vector.tensor_scalar(uf[:], t_sb[:], 1024.0, None,
                                    mybir.AluOpType.mult)
            nc.vector.tensor_copy(ii[:], uf[:])
            nc.vector.tensor_copy(fb[:], ii[:])
            nc.vector.tensor_tensor(adj[:], fb[:], uf[:], mybir.AluOpType.is_gt)
            nc.vector.tensor_sub(ui[:], fb[:], adj[:])
            nc.vector.tensor_scalar(idx[:], ui[:], 1023.0, None,
                                    mybir.AluOpType.min)
        else:
            # single fused op; valid iff the f32->i16 output convert truncates
            mx = cpool.tile([P, 1], f32)
            nc.vector.memset(mx[:], 1023.5)
            nc.vector.scalar_tensor_tensor(
                idx[:], t_sb[:], 1024.0, mx[:].to_broadcast([P, cols]),
                mybir.AluOpType.mult, mybir.AluOpType.min)

        dst = cpool.tile([P, S, GE], f32)
        og = cpool.tile([P, S, F], f32)
        nrounds = ninst // NQ
        for k in range(ninst):
            gi = _dma_gather_raw(
                nc.gpsimd, dst[:, k * spc:(k + 1) * spc, :],
                lut_d.ap()[:, 0:GE], idx[:, k * cpc:(k + 1) * cpc],
                ipc, GE, ROWW, k % NQ)
            gi.then_inc(qsems[k % NQ], 16)
        for r in range(nrounds):
            for q in range(NQ):
                nc.vector.wait_ge(qsems[q], 16 * (r + 1))
            sl = slice(r * spc * NQ, (r + 1) * spc * NQ)
            nc.vector.tensor_copy(og[:, sl, :], dst[:, sl, 0:F])
            nc.sync.dma_start(out_d.ap()[:, sl, :], og[:, sl, :])
    nc.compile()
    return nc


def _host_inputs(t, lut):
    tf = np.ascontiguousarray(np.asarray(t, np.float32)).reshape(-1)
    # wrapped layout: element j -> partition j%16, column j//16, x8 groups
    tw = tf.reshape(N_CORES, NPC // 16, 16).transpose(0, 2, 1)  # [m, 16, cols]
    tperm = np.tile(tw, (1, 8, 1))                              # [m, 128, cols]
    lutp = np.zeros((NBINS, ROWW), np.float32)
    lutp[:, :F] = lut
    return tperm, lutp


def kernel(t, W1, b1, W2, b2, W3, b3):
    global LAST_RESULTS
    key = ("nc", IMPL)
    if key not in _CACHE:
        assert IMPL.startswith("dg")
        spec = IMPL[2:]
        exact = spec.endswith("x")
        ninst = int(spec.rstrip("x") or 16)
        _CACHE[key] = _build_nc_dg(ninst, exact)
    nc = _CACHE[key]

    lut = _build_lut(np.asarray(W1, np.float32), np.asarray(b1, np.float32),
                     np.asarray(W2, np.float32), np.asarray(b2, np.float32),
                     np.asarray(W3, np.float32), np.asarray(b3, np.float32))
    tperm, lutp = _host_inputs(t, lut)
    in_maps = [{"t": np.ascontiguousarray(tperm[m]), "lut": lutp}
               for m in range(N_CORES)]

    res = run_bass_kernel_spmd(nc, in_maps, list(range(N_CORES)), **RUN_KWARGS)
    LAST_RESULTS = res
    # out[p, s] = element s*128 + p
    outs = [res.results[m]["out"].transpose(1, 0, 2).reshape(NPC, F)
            for m in range(N_CORES)]
    return np.concatenate(outs, axis=0).reshape(B, T, F).astype(np.float32)
